# revision 1
# baseline (speedup 1.0000x reference)
"""Trainium2 Bass kernel for BaseRelationNetwork forward pass.

Reference computation (per batch row b):
    pairs (i<j) of C=16 channels, P=120 pairs
    h1 = relu(concat(x_i, x_j) @ W1 + b1)      # W1 [2F, H]
    h2 = relu(h1 @ W2 + b2)
    out = mean_p(h2 @ W3 + b3)                 # [B, H]

Algebraic restructuring used here:
  1. W1 splits into W1a (top F rows, applied to x_i) and W1b (bottom F rows,
     applied to x_j). ya = x @ W1a and yb = x @ W1b are computed once per
     channel (C matmuls) instead of per pair (P matmuls): 7.5x less PE work.
     h1[p=(i,j)] = relu(ya[i] + yb[j] + b1) is a cheap DVE gather-add.
  2. mean over pairs commutes with the affine layer 3:
     out = (mean_p h2) @ W3 + b3. Layer 3 runs on the pair-mean only.

Sharding: data-parallel over batch. 512 rows / 8 cores = 64 rows per core.
Weights replicated. Host pre-transposes x to feature-major layout with
token = half*512 + c*32 + b (batch split in two halves of 32) so the
pipeline (layer-1 matmul -> pair-add -> layer-2 -> accumulate) runs as two
overlapping chunks; the 1/P mean scale is folded into W3 and the biases
are packed into one [128, 6] tile on the host.

Matmuls run in float32r (fast fp32 mode, reduced mantissa): full PE rate
when the moving free dim >= 256, ~1e-4 output error vs exact fp32.

DMA strategy: big loads (x, W1) go through gpsimd (SWDGE) as a few large
multi-tile transfers - the HWDGE queue serializes ~0.6us per dma_start, so
many small sync-engine DMAs throttle the front of the kernel.
"""

import contextlib
import sys

if "/opt/trn_rl_repo" not in sys.path:
    sys.path.insert(0, "/opt/trn_rl_repo")

import numpy as np

import concourse.bass as bass
import concourse.mybir as mybir
import concourse.tile as tile
from concourse import bacc
from concourse.bass_utils import run_bass_kernel_spmd

# Problem shape (hardcoded per contract).
B, C, F, H = 512, 16, 1024, 256
N_CORES = 8
BL = B // N_CORES          # 64 local batch rows per core
P = C * (C - 1) // 2       # 120 pairs
NH = 4                     # batch chunks per core (chunked pipeline)
BH = BL // NH              # 32 rows per half
TOK = BL * C               # 1024 tokens per core
HTOK = BH * C              # 512 tokens per half, token = half*512 + c*32 + b
F32 = mybir.dt.float32
F32R = mybir.dt.float32r

KT1 = F // 128             # 8 k-tiles for layer-1 contraction
KQ = 4                     # k-tiles per merged x DMA
PPG = 30                   # pairs per stage-C sub-group
GW = PPG * BH              # stage-C sub-group width: 480 columns
NG = P // PPG              # 8 stage-C sub-groups per half
NSP = NG // 2              # 4 double-width (960-col) stage-C groups per half

# pair enumeration: for i in 0..C-2, j in i+1..C-1, p consecutive
PAIR_BASE = [0] * C
for _i in range(1, C):
    PAIR_BASE[_i] = PAIR_BASE[_i - 1] + (C - 1 - (_i - 1))

AF = mybir.ActivationFunctionType
ALU = mybir.AluOpType


def build_module(loop_iters: int = 1):
    nc = bacc.Bacc("TRN2", target_bir_lowering=False, debug=True)

    xt_d = nc.dram_tensor("xt", [F, TOK], F32R, kind="ExternalInput")
    w1_d = nc.dram_tensor("w1", [2 * F, H], F32R, kind="ExternalInput")
    w2_d = nc.dram_tensor("w2", [H, H], F32R, kind="ExternalInput")
    w3_d = nc.dram_tensor("w3", [H, H], F32, kind="ExternalInput")
    bp_d = nc.dram_tensor("bias_pack", [128, 6], F32, kind="ExternalInput")
    id_d = nc.dram_tensor("ident", [128, 128], F32R, kind="ExternalInput")
    out_d = nc.dram_tensor("outT", [H, BL], F32, kind="ExternalOutput")

    with tile.TileContext(nc) as tc:
        with (
            tc.tile_pool(name="xpool", bufs=1) as xpool,
            tc.tile_pool(name="wpool", bufs=1) as wpool,
            tc.tile_pool(name="ypool", bufs=1) as ypool,
            tc.tile_pool(name="hpool", bufs=1) as hpool,
            tc.tile_pool(name="spool", bufs=1) as spool,
            tc.tile_pool(name="psA", bufs=4, space="PSUM") as psA_pool,
            tc.tile_pool(name="psC", bufs=2, space="PSUM") as psC_pool,
        ):
            loop_cm = (
                tc.For_i(0, loop_iters, 1)
                if loop_iters > 1
                else contextlib.nullcontext()
            )
            with loop_cm:
                # big tiles
                xts = xpool.tile([128, KT1, TOK], F32R, tag="xts", name="xts")
                w1big = wpool.tile([128, 2 * KT1, H], F32R, tag="w1big", name="w1big")
                w2t = wpool.tile([128, 2, H], F32R, tag="w2t", name="w2t")
                w3t = wpool.tile([128, 2, H], F32, tag="w3t", name="w3t")
                bp = wpool.tile([128, 6], F32, tag="bp", name="bp")
                idt = wpool.tile([128, 128], F32R, tag="idt", name="idt")
                # y_all free layout: [m(4), chunk(NH), c(C), b(BH)]
                y_all = ypool.tile([128, 4, TOK], F32, tag="y_all", name="y_all")
                # h1 free layout: [t(2), half(NH), p(P), b(BH)]
                h1all = hpool.tile(
                    [128, 2, NH * P * BH], F32R, tag="h1all", name="h1all"
                )
                h2sb = [
                    [
                        spool.tile(
                            [128, GW * NG], F32R,
                            tag=f"h2_{m}_{par}", name=f"h2_{m}_{par}",
                        )
                        for par in range(2)
                    ]
                    for m in range(2)
                ]
                m2 = [
                    spool.tile([128, BL], F32, tag=f"m2_{m}", name=f"m2_{m}")
                    for m in range(2)
                ]
                osb = spool.tile([128, 2, BL], F32, tag="osb", name="osb")

                def bias(nm, t):
                    idx = {"b1": 0, "b2": 2, "b3": 4}[nm] + t
                    return bp[:, idx : idx + 1]

                # W1 rows viewed [16 ktiles, 128, H] -> SBUF [128, k, H]
                w1v = w1_d.rearrange("(k p) h -> p k h", p=128)
                xtv = xt_d.rearrange("(k p) t -> p k t", p=128)

                def hs(half):
                    return slice(half * HTOK, (half + 1) * HTOK)

                # ---- DMA order: bias first (ya copies need b1), W1 quads +
                # x chunk 0, then w2 (stage C), remaining x chunks, w3 last ----
                nc.sync.dma_start(out=bp[:], in_=bp_d[:])
                nc.sync.dma_start(out=idt[:], in_=id_d[:])
                for q in range(2):
                    ks = slice(q * KQ, (q + 1) * KQ)
                    kbs = slice(KT1 + q * KQ, KT1 + (q + 1) * KQ)
                    nc.gpsimd.dma_start(out=w1big[:, ks, :], in_=w1v[:, ks, :])
                    nc.gpsimd.dma_start(out=w1big[:, kbs, :], in_=w1v[:, kbs, :])
                    nc.gpsimd.dma_start(
                        out=xts[:, ks, hs(0)], in_=xtv[:, ks, hs(0)]
                    )
                nc.sync.dma_start(
                    out=w2t[:], in_=w2_d.rearrange("(k p) h -> p k h", p=128)
                )
                for ch in range(1, NH):
                    nc.gpsimd.dma_start(
                        out=xts[:, :, hs(ch)], in_=xtv[:, :, hs(ch)]
                    )
                nc.sync.dma_start(
                    out=w3t[:], in_=w3_d.rearrange("(k p) h -> p k h", p=128)
                )

                def flush_acc(p):
                    ph, ppar = p
                    for m in range(2):
                        # sum the 4 su-blocks on PE: identity pass-through
                        # matmuls accumulating in PSUM (PE has slack)
                        psr = psC_pool.tile(
                            [128, GW], F32, tag="psC", name=f"psR_{ph}_{m}"
                        )
                        for su in range(2 * NSP):
                            nc.tensor.matmul(
                                psr[:],
                                idt[:],
                                h2sb[m][ppar][:, su * GW : (su + 1) * GW],
                                start=(su == 0),
                                stop=(su == 2 * NSP - 1),
                            )
                        # then reduce over p only: [128, b, p] view, 480 reads
                        v = psr.rearrange("q (pp b) -> q pp b", b=BH).transpose(
                            [0, 2, 1]
                        )
                        nc.vector.tensor_reduce(
                            m2[m][:, ph * BH : (ph + 1) * BH],
                            v,
                            mybir.AxisListType.X,
                            ALU.add,
                        )

                # PE warm-up while DMAs stream: ~10 dummy matmuls on the bias
                # tile into a psC-pool slot (free until stage C starts ~18us)
                warm = psC_pool.tile([128, 1024], F32, tag="psC", name="warm")
                for _ in range(10):
                    nc.tensor.matmul(
                        warm[:1, :256],
                        bp[:, 0:1],
                        bp[:, 0:1].broadcast_to([128, 256]),
                        start=True,
                        stop=True,
                    )

                pend = None
                for half in range(NH):
                    # ---- stage A (k-outer): matmuls for this half ----
                    psA = {
                        m: psA_pool.tile(
                            [128, HTOK], F32, tag="psA", name=f"psA_{half}_{m}"
                        )
                        for m in range(4)
                    }
                    for k in range(KT1):
                        for m in (0, 2, 1, 3):
                            w_half, ht = divmod(m, 2)
                            nc.tensor.matmul(
                                psA[m][:],
                                w1big[:, w_half * KT1 + k, ht * 128 : (ht + 1) * 128],
                                xts[:, k, hs(half)],
                                start=(k == 0),
                                stop=(k == KT1 - 1),
                            )
                    # PSUM -> SBUF copies, split DVE/ACT; b1 folded into ya
                    for m in (0, 2, 1, 3):
                        if m < 2:
                            nc.vector.tensor_scalar_add(
                                y_all[:, m, hs(half)], psA[m][:], bias("b1", m)
                            )
                        else:
                            nc.scalar.copy(y_all[:, m, hs(half)], psA[m][:])

                    # ---- stage B: pair-add + bias on DVE, relu on ACT ----
                    # y_all viewed [128, m, half, c, b]; h1all [128, t, half, p, b]
                    hbase = half * P * BH
                    y5 = y_all.rearrange("p m (hh c b) -> p m hh c b", hh=NH, b=BH)
                    h5 = h1all.rearrange("p t (hh pp b) -> p t hh pp b", hh=NH, b=BH)
                    for i in range(C - 1):
                        nj = C - 1 - i
                        p0 = PAIR_BASE[i]
                        in0 = y5[:, 0:2, half, i : i + 1, :].broadcast_to(
                            [128, 2, nj, BH]
                        )
                        in1 = y5[:, 2:4, half, i + 1 :, :]
                        outap = h5[:, :, half, p0 : p0 + nj, :]
                        nc.vector.tensor_add(outap, in0, in1)
                    # relu in place, both t at once, 960-wide slices (ACT)
                    for sp in range(NSP):
                        sl = h1all[
                            :, :, hbase + sp * 2 * GW : hbase + (sp + 1) * 2 * GW
                        ]
                        nc.scalar.activation(sl, sl, AF.Relu)

                    # flush the PREVIOUS chunk's DVE accumulate chain now, so
                    # this chunk's pair-adds (above) fed PE/ACT first
                    if pend is not None:
                        flush_acc(pend)
                        pend = None

                    # ---- stage C+D: layer-2 matmul, relu(+b2) on ACT ----
                    par = half % 2
                    for sp in range(NSP):
                        for m in range(2):
                            ps = psC_pool.tile(
                                [128, 1024], F32, tag="psC",
                                name=f"psC_{half}_{m}_{sp}",
                            )
                            for sub in range(2):
                                s = sp * 2 + sub
                                for k in range(2):
                                    nc.tensor.matmul(
                                        ps[:, sub * 512 : sub * 512 + GW],
                                        w2t[:, k, m * 128 : (m + 1) * 128],
                                        h1all[
                                            :,
                                            k,
                                            hbase + s * GW : hbase + (s + 1) * GW,
                                        ],
                                        start=(k == 0),
                                        stop=(k == 1),
                                    )
                            h2t = h2sb[m][par][
                                :, sp * 2 * GW : (sp + 1) * 2 * GW
                            ].rearrange("p (u g) -> p u g", g=GW)
                            psv = ps.rearrange("p (u g) -> p u g", g=512)[:, :, :GW]
                            nc.scalar.activation(h2t, psv, AF.Relu, bias=bias("b2", m))
                    pend = (half, par)

                if pend is not None:
                    flush_acc(pend)
                    pend = None

                # ---- stage E: outT = (m2 @ W3scaled) + b3 (bias on DVE) ----
                for mo in range(2):
                    ps = psA_pool.tile([128, HTOK], F32, tag="psA", name=f"psE_{mo}")
                    po = ps[:, :BL]
                    for k in range(2):
                        nc.tensor.matmul(
                            po,
                            w3t[:, k, mo * 128 : (mo + 1) * 128],
                            m2[k][:],
                            start=(k == 0),
                            stop=(k == 1),
                        )
                    nc.vector.tensor_scalar_add(osb[:, mo, :], po, bias("b3", mo))
                nc.sync.dma_start(
                    out=out_d.rearrange("(m p) b -> p m b", p=128), in_=osb[:]
                )

    nc.compile()
    return nc


_NC_CACHE = None


def _get_module():
    global _NC_CACHE
    if _NC_CACHE is None:
        _NC_CACHE = build_module()
    return _NC_CACHE


def make_in_maps(x, W1, b1, W2, b2, W3, b3):
    W1 = np.ascontiguousarray(W1, dtype=np.float32)
    w3p = np.ascontiguousarray(W3, dtype=np.float32) / np.float32(P)
    b1 = np.asarray(b1, dtype=np.float32)
    b2 = np.asarray(b2, dtype=np.float32)
    b3 = np.asarray(b3, dtype=np.float32)
    bias_pack = np.stack(
        [b1[:128], b1[128:], b2[:128], b2[128:], b3[:128], b3[128:]], axis=1
    )
    bias_pack = np.ascontiguousarray(bias_pack, dtype=np.float32)
    in_maps = []
    for i in range(N_CORES):
        xs = x[i * BL : (i + 1) * BL]  # [BL, C, F]
        halves = [
            xs[h * BH : (h + 1) * BH].transpose(1, 0, 2).reshape(HTOK, F)
            for h in range(NH)
        ]
        xT = np.ascontiguousarray(np.concatenate(halves, axis=0).T, dtype=np.float32)
        in_maps.append(
            {
                "xt": xT,
                "w1": W1,
                "w2": np.ascontiguousarray(W2, dtype=np.float32),
                "w3": np.ascontiguousarray(w3p, dtype=np.float32),
                "bias_pack": bias_pack,
                "ident": np.eye(128, dtype=np.float32),
            }
        )
    return in_maps


def kernel(x, W1, b1, W2, b2, W3, b3):
    nc = _get_module()
    in_maps = make_in_maps(
        np.asarray(x, dtype=np.float32),
        np.asarray(W1),
        np.asarray(b1),
        np.asarray(W2),
        np.asarray(b2),
        np.asarray(W3),
        np.asarray(b3),
    )
    res = run_bass_kernel_spmd(nc, in_maps, list(range(N_CORES)))
    out = np.empty((B, H), dtype=np.float32)
    for i in range(N_CORES):
        out[i * BL : (i + 1) * BL] = res.results[i]["outT"].T
    return out

